# revision 16
# baseline (speedup 1.0000x reference)
"""Trainium2 Bass kernel: causal self-attention with RoPE.

Problem: B=2, T=2048, C=1536, H=16 heads, D=96 head dim.
  qkv = x @ w_attn.T ; rope(q, k) ; causal softmax attention ; y = att @ w_proj.T

Sharding (8 cores): data-parallel over batch (2) x tensor-parallel over heads
(4 groups of 4 heads).  Each core computes, for its batch b and its 4 heads:
QKV projection, RoPE + causal attention, and its partial output projection
y_part = att_heads @ w_proj[:, cols].T; the 4 partials per batch are summed on
the host.

This version is a single merged pipeline (no phase barrier): for each
512-token quarter q we emit QKV+rope+transpose for that quarter, then the
attention for q-tile q (which only needs K/V up to quarter q), then the output
projection for q-tile q.  The Tile scheduler's per-engine ready heaps then
overlap attention's scalar/vector work (exp, normalize) with the next
quarter's QKV matmuls, keeping the PE continuously busy (no HAM re-throttle).

All matmul operands are bf16 (PSUM accumulation stays fp32).  The weights
carry 1/sqrt(C) scaling, so bf16 input rounding adds only ~0.2% noise to the
logits -- far inside the harness' 2e-2 gate.  bf16 also halves HBM traffic
and SBUF footprint, letting every pool stay open for the whole kernel.

PSUM budget (8 banks): qkv accumulators 3, transpose staging 1, S-block pair
2, PV accumulator 1, projection 1.
"""

import math

import numpy as np

import concourse.bass as bass
import concourse.mybir as mybir
import concourse.tile as tile
from concourse import bacc, bass_utils
from concourse.masks import make_identity

# ---------------------------------------------------------------- constants
B, T, C = 2, 2048, 1536
H, D = 16, 96
NCORES = 8
HPC = 4                      # heads per core
DH = HPC * D                 # 384 = per-core head-dim total
DH2 = HPC * (D // 2)         # 192 = per-core evens (or odds) width
SCALE = 1.0 / math.sqrt(D)
NT = T // 128                # 16 t-tiles of 128 tokens
NQ = T // 512                # 4 q-tiles of 512 queries
F32 = mybir.dt.float32
F32R = mybir.dt.float32r
BF16 = mybir.dt.bfloat16


def _qe(j, q0, qw=512):
    """Causally-live query start for key block j within the query window
    of width qw starting at absolute query q0."""
    return min(max(j * 128 - q0, 0), qw - 128)


# ---------------------------------------------------------------- device IR
def _build_kernel(reps=1):
    nc = bacc.Bacc(
        "TRN2",
        target_bir_lowering=False,
        debug=False,
        enable_asserts=False,
        num_devices=NCORES,
    )

    xT = nc.dram_tensor("xT", [C, T], BF16, kind="ExternalInput").ap()
    wqkvT = nc.dram_tensor("wqkvT", [C, 3 * DH], BF16, kind="ExternalInput").ap()
    wpT = nc.dram_tensor("wpT", [128, 3, C], BF16, kind="ExternalInput").ap()
    tab3 = nc.dram_tensor("tab3", [T, 3 * DH2], BF16, kind="ExternalInput").ap()
    tmd = nc.dram_tensor("tm", [128, 1024], BF16, kind="ExternalInput").ap()
    yp = nc.dram_tensor("yp", [T, C], BF16, kind="ExternalOutput").ap()

    with tile.TileContext(nc) as tc:
        for _ in range(reps):
            _body(tc, xT, wqkvT, wpT, tab3, tmd, yp)

    nc.compile()
    return nc


def _body(tc, xT, wqkvT, wpT, tab3, tmd, yp):
    nc = tc.nc
    Exp = mybir.ActivationFunctionType.Exp

    with (
        tc.tile_pool(name="persist", bufs=1) as persist,
        tc.tile_pool(name="pax", bufs=16) as pax,
        tc.tile_pool(name="ptab", bufs=2) as ptab,
        tc.tile_pool(name="pqt", bufs=2) as pqt,
        tc.tile_pool(name="par", bufs=1) as par,
        tc.tile_pool(name="prk", bufs=2) as prk,
        tc.tile_pool(name="ppt", bufs=3) as ppt,
        tc.tile_pool(name="pat", bufs=4) as pat,
        tc.tile_pool(name="pys", bufs=2) as pys,
        tc.tile_pool(name="pbr", bufs=2) as pbr,
        tc.tile_pool(name="pst", bufs=1, space="PSUM") as pst,
        tc.tile_pool(name="pacc", bufs=1, space="PSUM") as pacc,
        tc.tile_pool(name="pop", bufs=1, space="PSUM") as pop,
    ):
        # ---------------- persistent tiles --------------------------------
        KT = persist.tile([D, HPC, T], BF16)          # rope'd K^T
        V = persist.tile([128, HPC, NT, D + 1], BF16)  # V + ones col (denom)
        tm = persist.tile([128, 1024], BF16)           # causal mask
        wq_sb = persist.tile([128, 12, 3 * DH], BF16)
        wp_sb = persist.tile([128, 3, C], BF16)
        ident = persist.tile([128, 128], BF16)
        identf = persist.tile([128, 128], F32)
        onesf = persist.tile([128, D], F32)
        ones1 = persist.tile([1, D], F32R)

        # ---------------- setup + startup DMA -----------------------------
        # Sync ring: tab3 quarter 0, then x quarter 0 (interleaved with the
        # weight tiles on the scalar ring so the first matmul can start after
        # ~one tile of each).
        t3s = [None] * 4
        t3s[0] = ptab.tile([128, 4, 3 * DH2], BF16, name="t3_0", tag="t3")
        nc.sync.dma_start(
            out=t3s[0],
            in_=tab3[0:512, :].rearrange("(tt p) d -> p tt d", p=128),
        )
        xcs = {}
        for c in range(12):
            xc = pax.tile([128, 512], BF16, name=f"xc0_{c}", tag="xc")
            eng = nc.sync if c % 2 else nc.scalar
            eng.dma_start(out=xc, in_=xT[c * 128 : (c + 1) * 128, 0:512])
            weng = nc.scalar if c % 2 else nc.sync
            weng.dma_start(
                out=wq_sb[:, c, :], in_=wqkvT[c * 128 : (c + 1) * 128, :]
            )
            xcs[(0, c)] = xc
        nc.scalar.dma_start(out=wp_sb, in_=wpT)
        nc.sync.dma_start(out=tm, in_=tmd)
        make_identity(nc, identf)
        nc.scalar.copy(out=ident, in_=identf)
        # HAM warm-up: ~4us of dependency-free matmuls on the identity so
        # the PE clock is already 8/8 when the first weights land
        wu = pop.tile([128, 512], F32, name="wu", tag="op")

        def keepwarm(n):
            for r in range(n):
                nc.tensor.matmul(
                    wu[:, (r % 4) * 128 : (r % 4 + 1) * 128],
                    ident,
                    ident,
                    start=True,
                    stop=True,
                )

        keepwarm(16)
        nc.vector.memset(onesf, 1.0)
        nc.scalar.copy(
            out=V[:, :, :, D],
            in_=onesf[:, 0 : HPC * NT].rearrange("p (h t) -> p h t", h=HPC),
        )
        nc.scalar.copy(out=ones1, in_=onesf[0:1, :])

        QTs = [None] * 4
        attTs = [None] * 4

        def emit_prefetch(qn):
            t3s[qn] = ptab.tile(
                [128, 4, 3 * DH2], BF16, name=f"t3_{qn}", tag="t3"
            )
            nc.sync.dma_start(
                out=t3s[qn],
                in_=tab3[qn * 512 : (qn + 1) * 512, :].rearrange(
                    "(tt p) d -> p tt d", p=128
                ),
            )
            for c in range(12):
                xc = pax.tile([128, 512], BF16, name=f"xc{qn}_{c}", tag="xc")
                nc.sync.dma_start(
                    out=xc,
                    in_=xT[c * 128 : (c + 1) * 128, qn * 512 : (qn + 1) * 512],
                )
                xcs[(qn, c)] = xc

        def emit_quarter(q, ppq, ptp):
            if q < 3:
                emit_prefetch(q + 1)
            QT = pqt.tile([D, HPC, 512], BF16, name=f"QT{q}", tag="QT")
            QTs[q] = QT
            t3 = t3s[q]
            for tt in range(4):
                t0 = q * 4 + tt  # global 128-token tile index
                qp = ppq.tile([128, DH], F32, tag="qp")
                kp = ppq.tile([128, DH], F32, tag="kp")
                vp = ppq.tile([128, DH], F32, tag="vp")
                for c in range(12):
                    lhs = xcs[(q, c)][:, tt * 128 : (tt + 1) * 128]
                    w = wq_sb[:, c, :]
                    s0 = c == 0
                    s1 = c == 11
                    nc.tensor.matmul(qp, lhs, w[:, 0:DH], start=s0, stop=s1)
                    nc.tensor.matmul(
                        kp, lhs, w[:, DH : 2 * DH], start=s0, stop=s1
                    )
                    nc.tensor.matmul(
                        vp, lhs, w[:, 2 * DH : 3 * DH], start=s0, stop=s1
                    )
                    if q == 0 and tt == 0:
                        # keep the PE clock warm while the weight stream
                        # trickles in during the first accumulation
                        keepwarm(4)

                # V: [t, (h d)] -> V[:, h, t0, 0:D]
                nc.scalar.copy(
                    out=V[:, :, t0, 0:D],
                    in_=vp.rearrange("p (h d) -> p h d", h=HPC),
                )

                # rope: dst_even = e*c - o*s ; dst_odd = e*s + o*c
                # t3 layout: [s | c | -s]: [c|-s] = t3[192:576], [s|c] =
                # t3[0:384].  Muls on Vector, adds on GpSimd.
                qr = prk.tile([128, HPC, 2, D // 2], BF16, tag="qr")
                kr = prk.tile([128, HPC, 2, D // 2], BF16, tag="kr")
                for (src, dst, tag) in ((qp, qr, "q"), (kp, kr, "k")):
                    t12 = par.tile([128, DH], F32, tag=f"t12{tag}")
                    t34 = par.tile([128, DH], F32, tag=f"t34{tag}")
                    nc.vector.tensor_mul(t12, src, t3[:, tt, DH2 : DH2 + DH])
                    nc.vector.tensor_mul(t34, src, t3[:, tt, 0:DH])
                    nc.gpsimd.tensor_add(
                        dst[:, :, 0, :],
                        t12[:, 0:DH2].rearrange("p (h d) -> p h d", h=HPC),
                        t12[:, DH2:DH].rearrange("p (h d) -> p h d", h=HPC),
                    )
                    nc.gpsimd.tensor_add(
                        dst[:, :, 1, :],
                        t34[:, 0:DH2].rearrange("p (h d) -> p h d", h=HPC),
                        t34[:, DH2:DH].rearrange("p (h d) -> p h d", h=HPC),
                    )

                # transpose rope'd q/k tiles through one PSUM bank
                tpq = ptp.tile([D, HPC, 128], BF16, tag="tp")
                for h in range(HPC):
                    nc.tensor.transpose(tpq[:, h], qr[:, h], ident)
                nc.scalar.copy(
                    out=QT[:, :, tt * 128 : (tt + 1) * 128], in_=tpq
                )
                tpk = ptp.tile([D, HPC, 128], BF16, tag="tp")
                for h in range(HPC):
                    nc.tensor.transpose(tpk[:, h], kr[:, h], ident)
                nc.vector.tensor_copy(
                    KT[:, :, t0 * 128 : (t0 + 1) * 128], tpk
                )

        def emit_attn(i, stps, accs, lag=1):
            # q-tile i.  Software-pipelined: the PV pair for jp is emitted
            # `lag` pairs behind its S pair, so in the PE's (in-order)
            # instruction stream the next S pairs sit BEFORE a PV that is
            # still waiting on exp -- the PE never blocks on the ACT engine.
            q0 = i * 512
            QT = QTs[i]
            # attT in flat-(h,d) layout: row h*96+d -> [chunk r//128, r%128]
            attT = pat.tile([128, 3, 512], BF16, name=f"attT{i}", tag="attT")
            attTs[i] = attT
            nblk = 4 * i + 4
            jmax = nblk - 1
            npairs = nblk // 2
            for h in range(HPC):
                acc = accs[h % len(accs)].tile([D + 1, 512], F32, tag="acc")
                pts = {}

                def emit_s_exp(jp):
                    j0 = 2 * jp
                    stp = stps[jp % len(stps)].tile(
                        [128, 2, 512], F32, tag="stp"
                    )
                    for jj in range(2):
                        j = j0 + jj
                        qe = _qe(j, q0)
                        nc.tensor.matmul(
                            stp[:, jj, qe:],
                            KT[:, h, j * 128 : (j + 1) * 128],
                            QT[:, h, qe:512],
                            start=True,
                            stop=True,
                        )
                    pt = ppt.tile([128, 2, 512], BF16, tag="pt")
                    pts[jp] = pt
                    if _qe(j0, q0) > 0 or _qe(j0 + 1, q0) > 0:
                        # trimmed diagonal blocks: per-block exp over the
                        # causally-live range only
                        for jj in range(2):
                            qe = _qe(j0 + jj, q0)
                            nc.scalar.activation(
                                pt[:, jj, qe:],
                                stp[:, jj, qe:],
                                Exp,
                                scale=SCALE,
                            )
                    else:
                        nc.scalar.activation(
                            pt.rearrange("p a b -> p (a b)"),
                            stp.rearrange("p a b -> p (a b)"),
                            Exp,
                            scale=SCALE,
                        )
                    for jj in range(2):
                        j = j0 + jj
                        off = j * 128 - q0
                        if off >= 0:  # diagonal block: causal mask
                            qs = _qe(j, q0)
                            qf = off + 128
                            nc.gpsimd.tensor_mul(
                                pt[:, jj, qs:qf],
                                pt[:, jj, qs:qf],
                                tm[:, 512 - off + qs : 512 - off + qf],
                            )

                def emit_pv(jp):
                    j0 = 2 * jp
                    pt = pts.pop(jp)
                    for jj in range(2):
                        j = j0 + jj
                        qe = _qe(j, q0)
                        nc.tensor.matmul(
                            acc[:, qe:],
                            V[:, h, j],
                            pt[:, jj, qe:],
                            start=(j == 0),
                            stop=(j == jmax),
                        )

                for jp in range(npairs + lag):
                    if jp < npairs:
                        emit_s_exp(jp)
                    if jp >= lag:
                        emit_pv(jp - lag)

                # normalize: attT[:, h] = acc[0:D] * (1 / acc[D]) per col
                denS = pbr.tile([1, 512], F32, tag="denS")
                nc.scalar.copy(out=denS, in_=acc[D : D + 1, :])
                r1 = pbr.tile([1, 512], F32, tag="r1")
                nc.vector.reciprocal_approx_fast(r1, denS)
                r1r = pbr.tile([1, 512], F32R, tag="r1r")
                nc.vector.tensor_copy(r1r, r1)
                # broadcast 1/den across partitions via a K=1 matmul into a
                # reused stp slot; the normalize mul reads it from PSUM
                rept = stps[0].tile([128, 2, 512], F32, tag="stp")
                rep = rept[0:D, 0, :]
                nc.tensor.matmul(rep, ones1, r1r, start=True, stop=True)
                reps_t = pbr.tile([D, 512], F32, tag="reps")
                nc.vector.tensor_copy(reps_t, rep)
                # scatter the normalized rows into the flat-(h,d) layout,
                # split so each piece obeys the engine partition-alignment
                # rule (a pattern starting at partition 32/96 spans <= 32)
                def _allowed(b):
                    return 128 - b if b % 64 == 0 else 64 - b % 64

                r0 = h * D
                d0 = 0
                while d0 < D:
                    ch, row = (r0 + d0) // 128, (r0 + d0) % 128
                    dn = min(
                        D - d0, 128 - row, _allowed(d0 % 128), _allowed(row)
                    )
                    nc.vector.tensor_mul(
                        attT[row : row + dn, ch, :],
                        acc[d0 : d0 + dn, :],
                        reps_t[d0 : d0 + dn, :],
                    )
                    d0 += dn

        def emit_proj(i, pops=None, tts=range(4)):
            pops = pops or [pop]
            q0 = i * 512
            attT = attTs[i]
            for tt in tts:
                r0 = q0 + tt * 128
                ysb = pys.tile([128, C], BF16, tag="ysb")
                for os in range(3):
                    op = pops[(tt * 3 + os) % len(pops)].tile(
                        [128, 512], F32, tag="op"
                    )
                    for ch in range(3):
                        nc.tensor.matmul(
                            op,
                            attT[:, ch, tt * 128 : (tt + 1) * 128],
                            wp_sb[:, ch, os * 512 : (os + 1) * 512],
                            start=(ch == 0),
                            stop=(ch == 2),
                        )
                    if os == 0:
                        nc.scalar.copy(
                            out=ysb[:, os * 512 : (os + 1) * 512], in_=op
                        )
                    else:
                        nc.vector.tensor_copy(
                            ysb[:, os * 512 : (os + 1) * 512], op
                        )
                nc.sync.dma_start(out=yp[r0 : r0 + 128, :], in_=ysb)

        # ---------------- emission sequence -------------------------------
        # proj(i) is emitted after attn(i+1) so projection matmuls serve as
        # PE filler during the following attention's exp stalls; attn(3)
        # gets double-buffered S/acc PSUM from the banks the QKV
        # accumulators free after quarter 3.
        with (
            tc.tile_pool(name="ppq", bufs=1, space="PSUM") as ppq,
            tc.tile_pool(name="ptp", bufs=1, space="PSUM") as ptp,
        ):
            emit_quarter(0, ppq, ptp)
            emit_attn(0, [pst], [pacc], lag=1)
            emit_quarter(1, ppq, ptp)
            emit_attn(1, [pst], [pacc], lag=1)
            emit_proj(0, tts=(0, 1))
            emit_quarter(2, ppq, ptp)
            emit_attn(2, [pst], [pacc], lag=1)
            emit_proj(1, tts=(0, 1))
            emit_quarter(3, ppq, ptp)
        with (
            tc.tile_pool(name="pstB", bufs=1, space="PSUM") as pstB,
            tc.tile_pool(name="paccB", bufs=1, space="PSUM") as paccB,
            tc.tile_pool(name="popB", bufs=1, space="PSUM") as popB,
        ):
            emit_proj(0, [pop, popB], tts=(2, 3))
            emit_attn(3, [pst, pstB], [pacc, paccB], lag=2)
            emit_proj(2, [pop, popB])
            emit_proj(1, [pop, popB], tts=(2, 3))
            emit_proj(3, [pop, popB])


# ---------------------------------------------------------------- host side
def _rope_tables():
    inv_freq = 1.0 / (10000.0 ** (np.arange(0, D, 2, dtype=np.float32) / D))
    t = np.arange(T, dtype=np.float32)
    freqs = np.outer(t, inv_freq)                       # [T, 48]
    emb = np.concatenate([freqs, freqs], axis=-1)       # [T, 96]
    c = np.cos(emb)[:, ::2].astype(np.float32)          # [T, 48]
    s = np.sin(emb)[:, ::2].astype(np.float32)
    ct = np.ascontiguousarray(np.tile(c, (1, HPC)))     # [T, 192]
    st = np.ascontiguousarray(np.tile(s, (1, HPC)))
    # [s | c | -s]: [c|-s] = tab3[:, 192:576], [s|c] = tab3[:, 0:384]
    tab3 = np.ascontiguousarray(np.concatenate([st, ct, -st], axis=1))
    return tab3.astype(mybir.dt.np(BF16))


def _tri_mask():
    # tm[k, c] = 1.0 iff c >= k + 512
    k = np.arange(128)[:, None]
    c = np.arange(1024)[None, :]
    return (c >= k + 512).astype(mybir.dt.np(BF16))


def _core_inputs(x, w_attn, w_proj, core):
    b, g = divmod(core, HPC)
    heads = [HPC * g + hh for hh in range(HPC)]
    bf = mybir.dt.np(BF16)
    xTh = np.ascontiguousarray(x[b].T).astype(bf)       # [C, T]

    def rows(sec, h):
        return w_attn[sec * C + h * D : sec * C + (h + 1) * D]

    q_e = np.concatenate([rows(0, h)[0::2] for h in heads])   # [192, C]
    q_o = np.concatenate([rows(0, h)[1::2] for h in heads])
    k_e = np.concatenate([rows(1, h)[0::2] for h in heads])
    k_o = np.concatenate([rows(1, h)[1::2] for h in heads])
    v_r = np.concatenate([rows(2, h) for h in heads])         # [384, C]
    wqkv = np.concatenate([q_e, q_o, k_e, k_o, v_r])          # [1152, C]
    wqkvT = np.ascontiguousarray(wqkv.T).astype(bf)           # [C, 1152]

    wp_flat = np.concatenate(
        [w_proj[:, h * D : (h + 1) * D].T for h in heads]
    )                                                         # [384, C], (h,d)-major
    wpT = np.ascontiguousarray(
        wp_flat.reshape(3, 128, C).transpose(1, 0, 2)
    ).astype(bf)                                              # [128, 3, C]
    return {"xT": xTh, "wqkvT": wqkvT, "wpT": wpT}


_NC_CACHE = {}


def _get_nc(reps=1):
    if reps not in _NC_CACHE:
        _NC_CACHE[reps] = _build_kernel(reps)
    return _NC_CACHE[reps]


def make_in_maps(x, w_attn, w_proj):
    x = np.asarray(x, np.float32)
    w_attn = np.asarray(w_attn, np.float32)
    w_proj = np.asarray(w_proj, np.float32)
    tab3 = _rope_tables()
    tm = _tri_mask()
    in_maps = []
    for core in range(NCORES):
        m = _core_inputs(x, w_attn, w_proj, core)
        m["tab3"] = tab3
        m["tm"] = tm
        in_maps.append(m)
    return in_maps


def combine_outputs(results):
    y = np.empty((B, T, C), np.float32)
    for b in range(B):
        parts = [
            results[b * HPC + g]["yp"].astype(np.float32) for g in range(HPC)
        ]
        y[b] = parts[0] + parts[1] + parts[2] + parts[3]
    return y


def kernel(x, w_attn, w_proj, _trace=False, _trace_kwargs=None):
    nc = _get_nc()
    in_maps = make_in_maps(x, w_attn, w_proj)
    res = bass_utils.run_bass_kernel_spmd(
        nc,
        in_maps,
        core_ids=list(range(NCORES)),
        trace=_trace,
        **(_trace_kwargs or {}),
    )
    out = combine_outputs(res.results)
    if _trace:
        kernel._last_results = res
    return out


# revision 17
# speedup vs baseline: 1.0055x; 1.0055x over previous
"""Trainium2 Bass kernel: causal self-attention with RoPE.

Problem: B=2, T=2048, C=1536, H=16 heads, D=96 head dim.
  qkv = x @ w_attn.T ; rope(q, k) ; causal softmax attention ; y = att @ w_proj.T

Sharding (8 cores): data-parallel over batch (2) x tensor-parallel over heads
(4 groups of 4 heads).  Each core computes, for its batch b and its 4 heads:
QKV projection, RoPE + causal attention, and its partial output projection
y_part = att_heads @ w_proj[:, cols].T; the 4 partials per batch are summed on
the host.

This version is a single merged pipeline (no phase barrier): for each
512-token quarter q we emit QKV+rope+transpose for that quarter, then the
attention for q-tile q (which only needs K/V up to quarter q), then the output
projection for q-tile q.  The Tile scheduler's per-engine ready heaps then
overlap attention's scalar/vector work (exp, normalize) with the next
quarter's QKV matmuls, keeping the PE continuously busy (no HAM re-throttle).

All matmul operands are bf16 (PSUM accumulation stays fp32).  The weights
carry 1/sqrt(C) scaling, so bf16 input rounding adds only ~0.2% noise to the
logits -- far inside the harness' 2e-2 gate.  bf16 also halves HBM traffic
and SBUF footprint, letting every pool stay open for the whole kernel.

PSUM budget (8 banks): qkv accumulators 3, transpose staging 1, S-block pair
2, PV accumulator 1, projection 1.
"""

import math

import numpy as np

import concourse.bass as bass
import concourse.mybir as mybir
import concourse.tile as tile
from concourse import bacc, bass_utils
from concourse.masks import make_identity

# ---------------------------------------------------------------- constants
B, T, C = 2, 2048, 1536
H, D = 16, 96
NCORES = 8
HPC = 4                      # heads per core
DH = HPC * D                 # 384 = per-core head-dim total
DH2 = HPC * (D // 2)         # 192 = per-core evens (or odds) width
SCALE = 1.0 / math.sqrt(D)
NT = T // 128                # 16 t-tiles of 128 tokens
NQ = T // 512                # 4 q-tiles of 512 queries
F32 = mybir.dt.float32
F32R = mybir.dt.float32r
BF16 = mybir.dt.bfloat16


def _qe(j, q0, qw=512):
    """Causally-live query start for key block j within the query window
    of width qw starting at absolute query q0."""
    return min(max(j * 128 - q0, 0), qw - 128)


# ---------------------------------------------------------------- device IR
def _build_kernel(reps=1):
    nc = bacc.Bacc(
        "TRN2",
        target_bir_lowering=False,
        debug=False,
        enable_asserts=False,
        num_devices=NCORES,
    )

    xT = nc.dram_tensor("xT", [C, T], BF16, kind="ExternalInput").ap()
    wqkvT = nc.dram_tensor("wqkvT", [C, 3 * DH], BF16, kind="ExternalInput").ap()
    wpT = nc.dram_tensor("wpT", [128, 3, C], BF16, kind="ExternalInput").ap()
    tab3 = nc.dram_tensor("tab3", [T, 3 * DH2], BF16, kind="ExternalInput").ap()
    tmd = nc.dram_tensor("tm", [128, 1024], BF16, kind="ExternalInput").ap()
    yp = nc.dram_tensor("yp", [T, C], BF16, kind="ExternalOutput").ap()

    with tile.TileContext(nc) as tc:
        for _ in range(reps):
            _body(tc, xT, wqkvT, wpT, tab3, tmd, yp)

    nc.compile()
    return nc


def _body(tc, xT, wqkvT, wpT, tab3, tmd, yp):
    nc = tc.nc
    Exp = mybir.ActivationFunctionType.Exp

    with (
        tc.tile_pool(name="persist", bufs=1) as persist,
        tc.tile_pool(name="pax", bufs=16) as pax,
        tc.tile_pool(name="ptab", bufs=2) as ptab,
        tc.tile_pool(name="pqt", bufs=2) as pqt,
        tc.tile_pool(name="par", bufs=1) as par,
        tc.tile_pool(name="prk", bufs=2) as prk,
        tc.tile_pool(name="ppt", bufs=5) as ppt,
        tc.tile_pool(name="pat", bufs=4) as pat,
        tc.tile_pool(name="pys", bufs=2) as pys,
        tc.tile_pool(name="pbr", bufs=2) as pbr,
        tc.tile_pool(name="pst", bufs=1, space="PSUM") as pst,
        tc.tile_pool(name="pacc", bufs=1, space="PSUM") as pacc,
        tc.tile_pool(name="pop", bufs=1, space="PSUM") as pop,
    ):
        # ---------------- persistent tiles --------------------------------
        KT = persist.tile([D, HPC, T], BF16)          # rope'd K^T
        V = persist.tile([128, HPC, NT, D + 1], BF16)  # V + ones col (denom)
        tm = persist.tile([128, 1024], BF16)           # causal mask
        wq_sb = persist.tile([128, 12, 3 * DH], BF16)
        wp_sb = persist.tile([128, 3, C], BF16)
        ident = persist.tile([128, 128], BF16)
        identf = persist.tile([128, 128], F32)
        onesf = persist.tile([128, D], F32)
        ones1 = persist.tile([1, D], F32R)

        # ---------------- setup + startup DMA -----------------------------
        # Sync ring: tab3 quarter 0, then x quarter 0 (interleaved with the
        # weight tiles on the scalar ring so the first matmul can start after
        # ~one tile of each).
        t3s = [None] * 4
        t3s[0] = ptab.tile([128, 4, 3 * DH2], BF16, name="t3_0", tag="t3")
        nc.sync.dma_start(
            out=t3s[0],
            in_=tab3[0:512, :].rearrange("(tt p) d -> p tt d", p=128),
        )
        xcs = {}
        for c in range(12):
            xc = pax.tile([128, 512], BF16, name=f"xc0_{c}", tag="xc")
            eng = nc.sync if c % 2 else nc.scalar
            eng.dma_start(out=xc, in_=xT[c * 128 : (c + 1) * 128, 0:512])
            weng = nc.scalar if c % 2 else nc.sync
            weng.dma_start(
                out=wq_sb[:, c, :], in_=wqkvT[c * 128 : (c + 1) * 128, :]
            )
            xcs[(0, c)] = xc
        nc.scalar.dma_start(out=wp_sb, in_=wpT)
        nc.sync.dma_start(out=tm, in_=tmd)
        make_identity(nc, identf)
        nc.scalar.copy(out=ident, in_=identf)
        # HAM warm-up: ~4us of dependency-free matmuls on the identity so
        # the PE clock is already 8/8 when the first weights land
        wu = pop.tile([128, 512], F32, name="wu", tag="op")

        def keepwarm(n):
            for r in range(n):
                nc.tensor.matmul(
                    wu[:, (r % 4) * 128 : (r % 4 + 1) * 128],
                    ident,
                    ident,
                    start=True,
                    stop=True,
                )

        keepwarm(16)
        nc.vector.memset(onesf, 1.0)
        nc.scalar.copy(
            out=V[:, :, :, D],
            in_=onesf[:, 0 : HPC * NT].rearrange("p (h t) -> p h t", h=HPC),
        )
        nc.scalar.copy(out=ones1, in_=onesf[0:1, :])

        QTs = [None] * 4
        attTs = [None] * 4

        def emit_prefetch(qn):
            t3s[qn] = ptab.tile(
                [128, 4, 3 * DH2], BF16, name=f"t3_{qn}", tag="t3"
            )
            nc.sync.dma_start(
                out=t3s[qn],
                in_=tab3[qn * 512 : (qn + 1) * 512, :].rearrange(
                    "(tt p) d -> p tt d", p=128
                ),
            )
            for c in range(12):
                xc = pax.tile([128, 512], BF16, name=f"xc{qn}_{c}", tag="xc")
                nc.sync.dma_start(
                    out=xc,
                    in_=xT[c * 128 : (c + 1) * 128, qn * 512 : (qn + 1) * 512],
                )
                xcs[(qn, c)] = xc

        def emit_quarter(q, ppq, ptp):
            if q < 3:
                emit_prefetch(q + 1)
            QT = pqt.tile([D, HPC, 512], BF16, name=f"QT{q}", tag="QT")
            QTs[q] = QT
            t3 = t3s[q]
            for tt in range(4):
                t0 = q * 4 + tt  # global 128-token tile index
                qp = ppq.tile([128, DH], F32, tag="qp")
                kp = ppq.tile([128, DH], F32, tag="kp")
                vp = ppq.tile([128, DH], F32, tag="vp")
                for c in range(12):
                    lhs = xcs[(q, c)][:, tt * 128 : (tt + 1) * 128]
                    w = wq_sb[:, c, :]
                    s0 = c == 0
                    s1 = c == 11
                    nc.tensor.matmul(qp, lhs, w[:, 0:DH], start=s0, stop=s1)
                    nc.tensor.matmul(
                        kp, lhs, w[:, DH : 2 * DH], start=s0, stop=s1
                    )
                    nc.tensor.matmul(
                        vp, lhs, w[:, 2 * DH : 3 * DH], start=s0, stop=s1
                    )
                    if q == 0 and tt == 0:
                        # keep the PE clock warm while the weight stream
                        # trickles in during the first accumulation
                        keepwarm(4)

                # V: [t, (h d)] -> V[:, h, t0, 0:D]
                nc.scalar.copy(
                    out=V[:, :, t0, 0:D],
                    in_=vp.rearrange("p (h d) -> p h d", h=HPC),
                )

                # rope: dst_even = e*c - o*s ; dst_odd = e*s + o*c
                # t3 layout: [s | c | -s]: [c|-s] = t3[192:576], [s|c] =
                # t3[0:384].  Muls on Vector, adds on GpSimd.
                qr = prk.tile([128, HPC, 2, D // 2], BF16, tag="qr")
                kr = prk.tile([128, HPC, 2, D // 2], BF16, tag="kr")
                for (src, dst, tag) in ((qp, qr, "q"), (kp, kr, "k")):
                    t12 = par.tile([128, DH], F32, tag=f"t12{tag}")
                    t34 = par.tile([128, DH], F32, tag=f"t34{tag}")
                    nc.vector.tensor_mul(t12, src, t3[:, tt, DH2 : DH2 + DH])
                    nc.vector.tensor_mul(t34, src, t3[:, tt, 0:DH])
                    nc.gpsimd.tensor_add(
                        dst[:, :, 0, :],
                        t12[:, 0:DH2].rearrange("p (h d) -> p h d", h=HPC),
                        t12[:, DH2:DH].rearrange("p (h d) -> p h d", h=HPC),
                    )
                    nc.gpsimd.tensor_add(
                        dst[:, :, 1, :],
                        t34[:, 0:DH2].rearrange("p (h d) -> p h d", h=HPC),
                        t34[:, DH2:DH].rearrange("p (h d) -> p h d", h=HPC),
                    )

                # transpose rope'd q/k tiles through one PSUM bank
                tpq = ptp.tile([D, HPC, 128], BF16, tag="tp")
                for h in range(HPC):
                    nc.tensor.transpose(tpq[:, h], qr[:, h], ident)
                nc.scalar.copy(
                    out=QT[:, :, tt * 128 : (tt + 1) * 128], in_=tpq
                )
                tpk = ptp.tile([D, HPC, 128], BF16, tag="tp")
                for h in range(HPC):
                    nc.tensor.transpose(tpk[:, h], kr[:, h], ident)
                nc.vector.tensor_copy(
                    KT[:, :, t0 * 128 : (t0 + 1) * 128], tpk
                )

        def emit_attn(i, stps, accs, lag=1, hgroup=1):
            # q-tile i.  Software-pipelined: the PV pair for jp trails its S
            # pair by `lag` pairs, so in the PE's in-order stream the next S
            # pairs sit BEFORE a PV that is still waiting on exp.  With
            # hgroup=2, two heads' pair-streams interleave so one head's
            # normalize chain hides behind the other head's exp stream.
            q0 = i * 512
            QT = QTs[i]
            # attT in flat-(h,d) layout: row h*96+d -> [chunk r//128, r%128]
            attT = pat.tile([128, 3, 512], BF16, name=f"attT{i}", tag="attT")
            attTs[i] = attT
            nblk = 4 * i + 4
            jmax = nblk - 1
            npairs = nblk // 2
            pts = {}
            accd = {}

            def emit_s_exp(h, jp, pool):
                j0 = 2 * jp
                stp = pool.tile([128, 2, 512], F32, tag="stp")
                for jj in range(2):
                    j = j0 + jj
                    qe = _qe(j, q0)
                    nc.tensor.matmul(
                        stp[:, jj, qe:],
                        KT[:, h, j * 128 : (j + 1) * 128],
                        QT[:, h, qe:512],
                        start=True,
                        stop=True,
                    )
                pt = ppt.tile([128, 2, 512], BF16, tag="pt")
                pts[(h, jp)] = pt
                if _qe(j0, q0) > 0 or _qe(j0 + 1, q0) > 0:
                    # trimmed diagonal blocks: exp over the live range only
                    for jj in range(2):
                        qe = _qe(j0 + jj, q0)
                        nc.scalar.activation(
                            pt[:, jj, qe:], stp[:, jj, qe:], Exp, scale=SCALE
                        )
                else:
                    nc.scalar.activation(
                        pt.rearrange("p a b -> p (a b)"),
                        stp.rearrange("p a b -> p (a b)"),
                        Exp,
                        scale=SCALE,
                    )
                for jj in range(2):
                    j = j0 + jj
                    off = j * 128 - q0
                    if off >= 0:  # diagonal block: causal mask
                        qs = _qe(j, q0)
                        qf = off + 128
                        nc.gpsimd.tensor_mul(
                            pt[:, jj, qs:qf],
                            pt[:, jj, qs:qf],
                            tm[:, 512 - off + qs : 512 - off + qf],
                        )

            def emit_pv(h, jp):
                j0 = 2 * jp
                pt = pts.pop((h, jp))
                acc = accd[h]
                for jj in range(2):
                    j = j0 + jj
                    qe = _qe(j, q0)
                    nc.tensor.matmul(
                        acc[:, qe:],
                        V[:, h, j],
                        pt[:, jj, qe:],
                        start=(j == 0),
                        stop=(j == jmax),
                    )

            def normalize(h):
                # attT[:, h] = acc[0:D] * (1 / acc[D]) per column
                acc = accd[h]
                denS = pbr.tile([1, 512], F32, tag="denS")
                nc.scalar.copy(out=denS, in_=acc[D : D + 1, :])
                r1 = pbr.tile([1, 512], F32, tag="r1")
                nc.vector.reciprocal_approx_fast(r1, denS)
                r1r = pbr.tile([1, 512], F32R, tag="r1r")
                nc.vector.tensor_copy(r1r, r1)
                # broadcast 1/den across partitions via a K=1 matmul into a
                # reused stp slot
                rept = stps[0].tile([128, 2, 512], F32, tag="stp")
                rep = rept[0:D, 0, :]
                nc.tensor.matmul(rep, ones1, r1r, start=True, stop=True)
                reps_t = pbr.tile([D, 512], F32, tag="reps")
                nc.vector.tensor_copy(reps_t, rep)
                # scatter normalized rows into the flat-(h,d) layout, split
                # so each piece obeys the partition-alignment rule (a
                # pattern starting at partition 32/96 spans <= 32)
                def _allowed(b):
                    return 128 - b if b % 64 == 0 else 64 - b % 64

                r0 = h * D
                d0 = 0
                while d0 < D:
                    ch, row = (r0 + d0) // 128, (r0 + d0) % 128
                    dn = min(
                        D - d0, 128 - row, _allowed(d0 % 128), _allowed(row)
                    )
                    nc.vector.tensor_mul(
                        attT[row : row + dn, ch, :],
                        acc[d0 : d0 + dn, :],
                        reps_t[d0 : d0 + dn, :],
                    )
                    d0 += dn

            for hbase in range(0, HPC, hgroup):
                hs = list(range(hbase, hbase + hgroup))
                for k, h in enumerate(hs):
                    accd[h] = accs[k % len(accs)].tile(
                        [D + 1, 512], F32, name=f"acc{i}_{h}", tag="acc"
                    )
                for jp in range(npairs + lag):
                    for k, h in enumerate(hs):
                        if jp < npairs:
                            emit_s_exp(h, jp, stps[k % len(stps)])
                        if jp >= lag:
                            emit_pv(h, jp - lag)
                for h in hs:
                    normalize(h)

        def emit_proj(i, pops=None, tts=range(4)):
            pops = pops or [pop]
            q0 = i * 512
            attT = attTs[i]
            for tt in tts:
                r0 = q0 + tt * 128
                ysb = pys.tile([128, C], BF16, tag="ysb")
                for os in range(3):
                    op = pops[(tt * 3 + os) % len(pops)].tile(
                        [128, 512], F32, tag="op"
                    )
                    for ch in range(3):
                        nc.tensor.matmul(
                            op,
                            attT[:, ch, tt * 128 : (tt + 1) * 128],
                            wp_sb[:, ch, os * 512 : (os + 1) * 512],
                            start=(ch == 0),
                            stop=(ch == 2),
                        )
                    if os == 0:
                        nc.scalar.copy(
                            out=ysb[:, os * 512 : (os + 1) * 512], in_=op
                        )
                    else:
                        nc.vector.tensor_copy(
                            ysb[:, os * 512 : (os + 1) * 512], op
                        )
                nc.sync.dma_start(out=yp[r0 : r0 + 128, :], in_=ysb)

        # ---------------- emission sequence -------------------------------
        # proj(i) is emitted after attn(i+1) so projection matmuls serve as
        # PE filler during the following attention's exp stalls; attn(3)
        # gets double-buffered S/acc PSUM from the banks the QKV
        # accumulators free after quarter 3.
        with (
            tc.tile_pool(name="ppq", bufs=1, space="PSUM") as ppq,
            tc.tile_pool(name="ptp", bufs=1, space="PSUM") as ptp,
        ):
            emit_quarter(0, ppq, ptp)
            emit_attn(0, [pst], [pacc], lag=1)
            emit_quarter(1, ppq, ptp)
            emit_attn(1, [pst], [pacc], lag=1)
            emit_proj(0, tts=(0, 1))
            emit_quarter(2, ppq, ptp)
            emit_attn(2, [pst], [pacc], lag=1)
            emit_proj(1, tts=(0, 1))
            emit_quarter(3, ppq, ptp)
        with (
            tc.tile_pool(name="pstB", bufs=1, space="PSUM") as pstB,
            tc.tile_pool(name="paccB", bufs=1, space="PSUM") as paccB,
            tc.tile_pool(name="popB", bufs=1, space="PSUM") as popB,
        ):
            emit_proj(0, [pop, popB], tts=(2, 3))
            emit_attn(3, [pst, pstB], [pacc, paccB], lag=1, hgroup=2)
            emit_proj(2, [pop, popB])
            emit_proj(1, [pop, popB], tts=(2, 3))
            emit_proj(3, [pop, popB])


# ---------------------------------------------------------------- host side
def _rope_tables():
    inv_freq = 1.0 / (10000.0 ** (np.arange(0, D, 2, dtype=np.float32) / D))
    t = np.arange(T, dtype=np.float32)
    freqs = np.outer(t, inv_freq)                       # [T, 48]
    emb = np.concatenate([freqs, freqs], axis=-1)       # [T, 96]
    c = np.cos(emb)[:, ::2].astype(np.float32)          # [T, 48]
    s = np.sin(emb)[:, ::2].astype(np.float32)
    ct = np.ascontiguousarray(np.tile(c, (1, HPC)))     # [T, 192]
    st = np.ascontiguousarray(np.tile(s, (1, HPC)))
    # [s | c | -s]: [c|-s] = tab3[:, 192:576], [s|c] = tab3[:, 0:384]
    tab3 = np.ascontiguousarray(np.concatenate([st, ct, -st], axis=1))
    return tab3.astype(mybir.dt.np(BF16))


def _tri_mask():
    # tm[k, c] = 1.0 iff c >= k + 512
    k = np.arange(128)[:, None]
    c = np.arange(1024)[None, :]
    return (c >= k + 512).astype(mybir.dt.np(BF16))


def _core_inputs(x, w_attn, w_proj, core):
    b, g = divmod(core, HPC)
    heads = [HPC * g + hh for hh in range(HPC)]
    bf = mybir.dt.np(BF16)
    xTh = np.ascontiguousarray(x[b].T).astype(bf)       # [C, T]

    def rows(sec, h):
        return w_attn[sec * C + h * D : sec * C + (h + 1) * D]

    q_e = np.concatenate([rows(0, h)[0::2] for h in heads])   # [192, C]
    q_o = np.concatenate([rows(0, h)[1::2] for h in heads])
    k_e = np.concatenate([rows(1, h)[0::2] for h in heads])
    k_o = np.concatenate([rows(1, h)[1::2] for h in heads])
    v_r = np.concatenate([rows(2, h) for h in heads])         # [384, C]
    wqkv = np.concatenate([q_e, q_o, k_e, k_o, v_r])          # [1152, C]
    wqkvT = np.ascontiguousarray(wqkv.T).astype(bf)           # [C, 1152]

    wp_flat = np.concatenate(
        [w_proj[:, h * D : (h + 1) * D].T for h in heads]
    )                                                         # [384, C], (h,d)-major
    wpT = np.ascontiguousarray(
        wp_flat.reshape(3, 128, C).transpose(1, 0, 2)
    ).astype(bf)                                              # [128, 3, C]
    return {"xT": xTh, "wqkvT": wqkvT, "wpT": wpT}


_NC_CACHE = {}


def _get_nc(reps=1):
    if reps not in _NC_CACHE:
        _NC_CACHE[reps] = _build_kernel(reps)
    return _NC_CACHE[reps]


def make_in_maps(x, w_attn, w_proj):
    x = np.asarray(x, np.float32)
    w_attn = np.asarray(w_attn, np.float32)
    w_proj = np.asarray(w_proj, np.float32)
    tab3 = _rope_tables()
    tm = _tri_mask()
    in_maps = []
    for core in range(NCORES):
        m = _core_inputs(x, w_attn, w_proj, core)
        m["tab3"] = tab3
        m["tm"] = tm
        in_maps.append(m)
    return in_maps


def combine_outputs(results):
    y = np.empty((B, T, C), np.float32)
    for b in range(B):
        parts = [
            results[b * HPC + g]["yp"].astype(np.float32) for g in range(HPC)
        ]
        y[b] = parts[0] + parts[1] + parts[2] + parts[3]
    return y


def kernel(x, w_attn, w_proj, _trace=False, _trace_kwargs=None):
    nc = _get_nc()
    in_maps = make_in_maps(x, w_attn, w_proj)
    res = bass_utils.run_bass_kernel_spmd(
        nc,
        in_maps,
        core_ids=list(range(NCORES)),
        trace=_trace,
        **(_trace_kwargs or {}),
    )
    out = combine_outputs(res.results)
    if _trace:
        kernel._last_results = res
    return out


# revision 18
# speedup vs baseline: 1.0361x; 1.0304x over previous
"""Trainium2 Bass kernel: causal self-attention with RoPE.

Problem: B=2, T=2048, C=1536, H=16 heads, D=96 head dim.
  qkv = x @ w_attn.T ; rope(q, k) ; causal softmax attention ; y = att @ w_proj.T

Sharding (8 cores): data-parallel over batch (2) x tensor-parallel over heads
(4 groups of 4 heads).  Each core computes, for its batch b and its 4 heads:
QKV projection, RoPE + causal attention, and its partial output projection
y_part = att_heads @ w_proj[:, cols].T; the 4 partials per batch are summed on
the host.

This version is a single merged pipeline (no phase barrier): for each
512-token quarter q we emit QKV+rope+transpose for that quarter, then the
attention for q-tile q (which only needs K/V up to quarter q), then the output
projection for q-tile q.  The Tile scheduler's per-engine ready heaps then
overlap attention's scalar/vector work (exp, normalize) with the next
quarter's QKV matmuls, keeping the PE continuously busy (no HAM re-throttle).

All matmul operands are bf16 (PSUM accumulation stays fp32).  The weights
carry 1/sqrt(C) scaling, so bf16 input rounding adds only ~0.2% noise to the
logits -- far inside the harness' 2e-2 gate.  bf16 also halves HBM traffic
and SBUF footprint, letting every pool stay open for the whole kernel.

PSUM budget (8 banks): qkv accumulators 3, transpose staging 1, S-block pair
2, PV accumulator 1, projection 1.
"""

import math

import numpy as np

import concourse.bass as bass
import concourse.mybir as mybir
import concourse.tile as tile
from concourse import bacc, bass_utils
from concourse.masks import make_identity

# ---------------------------------------------------------------- constants
B, T, C = 2, 2048, 1536
H, D = 16, 96
NCORES = 8
HPC = 4                      # heads per core
DH = HPC * D                 # 384 = per-core head-dim total
DH2 = HPC * (D // 2)         # 192 = per-core evens (or odds) width
SCALE = 1.0 / math.sqrt(D)
NT = T // 128                # 16 t-tiles of 128 tokens
NQ = T // 512                # 4 q-tiles of 512 queries
F32 = mybir.dt.float32
F32R = mybir.dt.float32r
BF16 = mybir.dt.bfloat16


def _qe(j, q0, qw=512):
    """Causally-live query start for key block j within the query window
    of width qw starting at absolute query q0."""
    return min(max(j * 128 - q0, 0), qw - 128)


# ---------------------------------------------------------------- device IR
def _build_kernel(reps=1):
    nc = bacc.Bacc(
        "TRN2",
        target_bir_lowering=False,
        debug=False,
        enable_asserts=False,
        num_devices=NCORES,
    )

    xT = nc.dram_tensor("xT", [C, T], BF16, kind="ExternalInput").ap()
    wqkvT = nc.dram_tensor("wqkvT", [C, 3 * DH], BF16, kind="ExternalInput").ap()
    wpT = nc.dram_tensor("wpT", [128, 3, C], BF16, kind="ExternalInput").ap()
    tab3 = nc.dram_tensor("tab3", [T, 3 * DH2], BF16, kind="ExternalInput").ap()
    tmd = nc.dram_tensor("tm", [128, 1024], BF16, kind="ExternalInput").ap()
    yp = nc.dram_tensor("yp", [T, C], BF16, kind="ExternalOutput").ap()

    with tile.TileContext(nc) as tc:
        for _ in range(reps):
            _body(tc, xT, wqkvT, wpT, tab3, tmd, yp)

    nc.compile()
    return nc


def _body(tc, xT, wqkvT, wpT, tab3, tmd, yp):
    nc = tc.nc
    Exp = mybir.ActivationFunctionType.Exp

    with (
        tc.tile_pool(name="persist", bufs=1) as persist,
        tc.tile_pool(name="pax", bufs=16) as pax,
        tc.tile_pool(name="ptab", bufs=2) as ptab,
        tc.tile_pool(name="pqt", bufs=2) as pqt,
        tc.tile_pool(name="par", bufs=1) as par,
        tc.tile_pool(name="prk", bufs=2) as prk,
        tc.tile_pool(name="ppt", bufs=5) as ppt,
        tc.tile_pool(name="pat", bufs=4) as pat,
        tc.tile_pool(name="pys", bufs=2) as pys,
        tc.tile_pool(name="pbr", bufs=2) as pbr,
        tc.tile_pool(name="pst", bufs=1, space="PSUM") as pst,
        tc.tile_pool(name="pacc", bufs=1, space="PSUM") as pacc,
        tc.tile_pool(name="pop", bufs=1, space="PSUM") as pop,
    ):
        # ---------------- persistent tiles --------------------------------
        KT = persist.tile([D, HPC, T], BF16)          # rope'd K^T
        V = persist.tile([128, HPC, NT, D + 1], BF16)  # V + ones col (denom)
        tm = persist.tile([128, 1024], BF16)           # causal mask
        wq_sb = persist.tile([128, 12, 3 * DH], BF16)
        wp_sb = persist.tile([128, 3, C], BF16)
        ident = persist.tile([128, 128], BF16)
        identf = persist.tile([128, 128], F32)
        onesf = persist.tile([128, D], F32)
        ones1 = persist.tile([1, D], F32R)

        # ---------------- setup + startup DMA -----------------------------
        # Sync ring: tab3 quarter 0, then x quarter 0 (interleaved with the
        # weight tiles on the scalar ring so the first matmul can start after
        # ~one tile of each).
        t3s = [None] * 4
        t3s[0] = ptab.tile([128, 4, 3 * DH2], BF16, name="t3_0", tag="t3")
        nc.sync.dma_start(
            out=t3s[0],
            in_=tab3[0:512, :].rearrange("(tt p) d -> p tt d", p=128),
        )
        xcs = {}
        for c in range(12):
            xc = pax.tile([128, 512], BF16, name=f"xc0_{c}", tag="xc")
            eng = nc.sync if c % 2 else nc.scalar
            eng.dma_start(out=xc, in_=xT[c * 128 : (c + 1) * 128, 0:512])
            weng = nc.scalar if c % 2 else nc.sync
            weng.dma_start(
                out=wq_sb[:, c, :], in_=wqkvT[c * 128 : (c + 1) * 128, :]
            )
            xcs[(0, c)] = xc
        nc.scalar.dma_start(out=wp_sb, in_=wpT)
        nc.sync.dma_start(out=tm, in_=tmd)
        make_identity(nc, identf)
        nc.scalar.copy(out=ident, in_=identf)
        # HAM warm-up: ~4us of dependency-free matmuls on the identity so
        # the PE clock is already 8/8 when the first weights land
        wu = pop.tile([128, 512], F32, name="wu", tag="op")

        def keepwarm(n):
            for r in range(n):
                nc.tensor.matmul(
                    wu[:, (r % 4) * 128 : (r % 4 + 1) * 128],
                    ident,
                    ident,
                    start=True,
                    stop=True,
                )

        keepwarm(16)
        nc.vector.memset(onesf, 1.0)
        nc.scalar.copy(
            out=V[:, :, :, D],
            in_=onesf[:, 0 : HPC * NT].rearrange("p (h t) -> p h t", h=HPC),
        )
        nc.scalar.copy(out=ones1, in_=onesf[0:1, :])

        QTs = [None] * 4
        attTs = [None] * 4

        def emit_prefetch(qn):
            t3s[qn] = ptab.tile(
                [128, 4, 3 * DH2], BF16, name=f"t3_{qn}", tag="t3"
            )
            nc.sync.dma_start(
                out=t3s[qn],
                in_=tab3[qn * 512 : (qn + 1) * 512, :].rearrange(
                    "(tt p) d -> p tt d", p=128
                ),
            )
            for c in range(12):
                xc = pax.tile([128, 512], BF16, name=f"xc{qn}_{c}", tag="xc")
                nc.sync.dma_start(
                    out=xc,
                    in_=xT[c * 128 : (c + 1) * 128, qn * 512 : (qn + 1) * 512],
                )
                xcs[(qn, c)] = xc

        def emit_quarter(q, ppq, ptp):
            if q < 3:
                emit_prefetch(q + 1)
            QT = pqt.tile([D, HPC, 512], BF16, name=f"QT{q}", tag="QT")
            QTs[q] = QT
            t3 = t3s[q]
            for tt in range(4):
                t0 = q * 4 + tt  # global 128-token tile index
                qp = ppq.tile([128, DH], F32, tag="qp")
                kp = ppq.tile([128, DH], F32, tag="kp")
                vp = ppq.tile([128, DH], F32, tag="vp")
                for c in range(12):
                    lhs = xcs[(q, c)][:, tt * 128 : (tt + 1) * 128]
                    w = wq_sb[:, c, :]
                    s0 = c == 0
                    s1 = c == 11
                    nc.tensor.matmul(qp, lhs, w[:, 0:DH], start=s0, stop=s1)
                    nc.tensor.matmul(
                        kp, lhs, w[:, DH : 2 * DH], start=s0, stop=s1
                    )
                    nc.tensor.matmul(
                        vp, lhs, w[:, 2 * DH : 3 * DH], start=s0, stop=s1
                    )
                    if q == 0 and tt == 0:
                        # keep the PE clock warm while the weight stream
                        # trickles in during the first accumulation
                        keepwarm(4)

                # V: [t, (h d)] -> V[:, h, t0, 0:D]
                nc.scalar.copy(
                    out=V[:, :, t0, 0:D],
                    in_=vp.rearrange("p (h d) -> p h d", h=HPC),
                )

                # rope: dst_even = e*c - o*s ; dst_odd = e*s + o*c
                # t3 layout: [s | c | -s]: [c|-s] = t3[192:576], [s|c] =
                # t3[0:384].  Muls on Vector, adds on GpSimd.
                qr = prk.tile([128, HPC, 2, D // 2], BF16, tag="qr")
                kr = prk.tile([128, HPC, 2, D // 2], BF16, tag="kr")
                for (src, dst, tag) in ((qp, qr, "q"), (kp, kr, "k")):
                    t12 = par.tile([128, DH], F32, tag=f"t12{tag}")
                    t34 = par.tile([128, DH], F32, tag=f"t34{tag}")
                    nc.vector.tensor_mul(t12, src, t3[:, tt, DH2 : DH2 + DH])
                    nc.vector.tensor_mul(t34, src, t3[:, tt, 0:DH])
                    nc.gpsimd.tensor_add(
                        dst[:, :, 0, :],
                        t12[:, 0:DH2].rearrange("p (h d) -> p h d", h=HPC),
                        t12[:, DH2:DH].rearrange("p (h d) -> p h d", h=HPC),
                    )
                    nc.gpsimd.tensor_add(
                        dst[:, :, 1, :],
                        t34[:, 0:DH2].rearrange("p (h d) -> p h d", h=HPC),
                        t34[:, DH2:DH].rearrange("p (h d) -> p h d", h=HPC),
                    )

                # transpose rope'd q/k tiles through one PSUM bank
                tpq = ptp.tile([D, HPC, 128], BF16, tag="tp")
                for h in range(HPC):
                    nc.tensor.transpose(tpq[:, h], qr[:, h], ident)
                nc.scalar.copy(
                    out=QT[:, :, tt * 128 : (tt + 1) * 128], in_=tpq
                )
                tpk = ptp.tile([D, HPC, 128], BF16, tag="tp")
                for h in range(HPC):
                    nc.tensor.transpose(tpk[:, h], kr[:, h], ident)
                nc.vector.tensor_copy(
                    KT[:, :, t0 * 128 : (t0 + 1) * 128], tpk
                )

        def emit_attn(i, stps, accs, lag=1, hgroup=1, den_vec=False):
            # q-tile i.  Software-pipelined: the PV pair for jp trails its S
            # pair by `lag` pairs, so in the PE's in-order stream the next S
            # pairs sit BEFORE a PV that is still waiting on exp.  With
            # hgroup=2, two heads' pair-streams interleave so one head's
            # normalize chain hides behind the other head's exp stream.
            q0 = i * 512
            QT = QTs[i]
            # attT in flat-(h,d) layout: row h*96+d -> [chunk r//128, r%128]
            attT = pat.tile([128, 3, 512], BF16, name=f"attT{i}", tag="attT")
            attTs[i] = attT
            nblk = 4 * i + 4
            jmax = nblk - 1
            npairs = nblk // 2
            pts = {}
            accd = {}

            def emit_s_exp(h, jp, pool):
                j0 = 2 * jp
                stp = pool.tile([128, 2, 512], F32, tag="stp")
                for jj in range(2):
                    j = j0 + jj
                    qe = _qe(j, q0)
                    nc.tensor.matmul(
                        stp[:, jj, qe:],
                        KT[:, h, j * 128 : (j + 1) * 128],
                        QT[:, h, qe:512],
                        start=True,
                        stop=True,
                    )
                pt = ppt.tile([128, 2, 512], BF16, tag="pt")
                pts[(h, jp)] = pt
                if _qe(j0, q0) > 0 or _qe(j0 + 1, q0) > 0:
                    # trimmed diagonal blocks: exp over the live range only
                    for jj in range(2):
                        qe = _qe(j0 + jj, q0)
                        nc.scalar.activation(
                            pt[:, jj, qe:], stp[:, jj, qe:], Exp, scale=SCALE
                        )
                else:
                    nc.scalar.activation(
                        pt.rearrange("p a b -> p (a b)"),
                        stp.rearrange("p a b -> p (a b)"),
                        Exp,
                        scale=SCALE,
                    )
                for jj in range(2):
                    j = j0 + jj
                    off = j * 128 - q0
                    if off >= 0:  # diagonal block: causal mask
                        qs = _qe(j, q0)
                        qf = off + 128
                        nc.gpsimd.tensor_mul(
                            pt[:, jj, qs:qf],
                            pt[:, jj, qs:qf],
                            tm[:, 512 - off + qs : 512 - off + qf],
                        )

            def emit_pv(h, jp):
                j0 = 2 * jp
                pt = pts.pop((h, jp))
                acc = accd[h]
                for jj in range(2):
                    j = j0 + jj
                    qe = _qe(j, q0)
                    nc.tensor.matmul(
                        acc[:, qe:],
                        V[:, h, j],
                        pt[:, jj, qe:],
                        start=(j == 0),
                        stop=(j == jmax),
                    )

            def normalize(h):
                # attT[:, h] = acc[0:D] * (1 / acc[D]) per column
                acc = accd[h]
                denS = pbr.tile([1, 512], F32, tag="denS")
                if den_vec:
                    nc.vector.tensor_copy(denS, acc[D : D + 1, :])
                else:
                    nc.scalar.copy(out=denS, in_=acc[D : D + 1, :])
                r1 = pbr.tile([1, 512], F32, tag="r1")
                nc.vector.reciprocal_approx_fast(r1, denS)
                r1r = pbr.tile([1, 512], F32R, tag="r1r")
                nc.vector.tensor_copy(r1r, r1)
                # broadcast 1/den across partitions via a K=1 matmul into a
                # reused stp slot
                rept = stps[0].tile([128, 2, 512], F32, tag="stp")
                rep = rept[0:D, 0, :]
                nc.tensor.matmul(rep, ones1, r1r, start=True, stop=True)
                reps_t = pbr.tile([D, 512], F32, tag="reps")
                nc.vector.tensor_copy(reps_t, rep)
                # scatter normalized rows into the flat-(h,d) layout, split
                # so each piece obeys the partition-alignment rule (a
                # pattern starting at partition 32/96 spans <= 32)
                def _allowed(b):
                    return 128 - b if b % 64 == 0 else 64 - b % 64

                r0 = h * D
                d0 = 0
                while d0 < D:
                    ch, row = (r0 + d0) // 128, (r0 + d0) % 128
                    dn = min(
                        D - d0, 128 - row, _allowed(d0 % 128), _allowed(row)
                    )
                    nc.vector.tensor_mul(
                        attT[row : row + dn, ch, :],
                        acc[d0 : d0 + dn, :],
                        reps_t[d0 : d0 + dn, :],
                    )
                    d0 += dn

            for hbase in range(0, HPC, hgroup):
                hs = list(range(hbase, hbase + hgroup))
                for k, h in enumerate(hs):
                    accd[h] = accs[k % len(accs)].tile(
                        [D + 1, 512], F32, name=f"acc{i}_{h}", tag="acc"
                    )
                for jp in range(npairs + lag):
                    for k, h in enumerate(hs):
                        if jp < npairs:
                            emit_s_exp(h, jp, stps[k % len(stps)])
                        if jp >= lag:
                            emit_pv(h, jp - lag)
                for h in hs:
                    normalize(h)

        def emit_proj(i, pops=None, tts=range(4), mix_copy=False):
            pops = pops or [pop]
            q0 = i * 512
            attT = attTs[i]
            for tt in tts:
                r0 = q0 + tt * 128
                ysb = pys.tile([128, C], BF16, tag="ysb")
                for os in range(3):
                    op = pops[(tt * 3 + os) % len(pops)].tile(
                        [128, 512], F32, tag="op"
                    )
                    for ch in range(3):
                        nc.tensor.matmul(
                            op,
                            attT[:, ch, tt * 128 : (tt + 1) * 128],
                            wp_sb[:, ch, os * 512 : (os + 1) * 512],
                            start=(ch == 0),
                            stop=(ch == 2),
                        )
                    if mix_copy and os == 0:
                        nc.scalar.copy(
                            out=ysb[:, os * 512 : (os + 1) * 512], in_=op
                        )
                    else:
                        nc.vector.tensor_copy(
                            ysb[:, os * 512 : (os + 1) * 512], op
                        )
                nc.sync.dma_start(out=yp[r0 : r0 + 128, :], in_=ysb)

        # ---------------- emission sequence -------------------------------
        # proj(i) is emitted after attn(i+1) so projection matmuls serve as
        # PE filler during the following attention's exp stalls; attn(3)
        # gets double-buffered S/acc PSUM from the banks the QKV
        # accumulators free after quarter 3.
        with (
            tc.tile_pool(name="ppq", bufs=1, space="PSUM") as ppq,
            tc.tile_pool(name="ptp", bufs=1, space="PSUM") as ptp,
        ):
            emit_quarter(0, ppq, ptp)
            emit_attn(0, [pst], [pacc], lag=1)
            emit_quarter(1, ppq, ptp)
            emit_attn(1, [pst], [pacc], lag=1)
            emit_proj(0, tts=(0, 1))
            emit_quarter(2, ppq, ptp)
            emit_attn(2, [pst], [pacc], lag=1)
            emit_proj(1, tts=(0, 1))
            emit_quarter(3, ppq, ptp)
        with (
            tc.tile_pool(name="pstB", bufs=1, space="PSUM") as pstB,
            tc.tile_pool(name="paccB", bufs=1, space="PSUM") as paccB,
            tc.tile_pool(name="popB", bufs=1, space="PSUM") as popB,
        ):
            emit_proj(0, [pop, popB], tts=(2, 3))
            emit_attn(3, [pst, pstB], [pacc, paccB], lag=1, hgroup=2,
                      den_vec=True)
            emit_proj(2, [pop, popB])
            emit_proj(1, [pop, popB], tts=(2, 3), mix_copy=True)
            emit_proj(3, [pop, popB], mix_copy=True)


# ---------------------------------------------------------------- host side
def _rope_tables():
    inv_freq = 1.0 / (10000.0 ** (np.arange(0, D, 2, dtype=np.float32) / D))
    t = np.arange(T, dtype=np.float32)
    freqs = np.outer(t, inv_freq)                       # [T, 48]
    emb = np.concatenate([freqs, freqs], axis=-1)       # [T, 96]
    c = np.cos(emb)[:, ::2].astype(np.float32)          # [T, 48]
    s = np.sin(emb)[:, ::2].astype(np.float32)
    ct = np.ascontiguousarray(np.tile(c, (1, HPC)))     # [T, 192]
    st = np.ascontiguousarray(np.tile(s, (1, HPC)))
    # [s | c | -s]: [c|-s] = tab3[:, 192:576], [s|c] = tab3[:, 0:384]
    tab3 = np.ascontiguousarray(np.concatenate([st, ct, -st], axis=1))
    return tab3.astype(mybir.dt.np(BF16))


def _tri_mask():
    # tm[k, c] = 1.0 iff c >= k + 512
    k = np.arange(128)[:, None]
    c = np.arange(1024)[None, :]
    return (c >= k + 512).astype(mybir.dt.np(BF16))


def _core_inputs(x, w_attn, w_proj, core):
    b, g = divmod(core, HPC)
    heads = [HPC * g + hh for hh in range(HPC)]
    bf = mybir.dt.np(BF16)
    xTh = np.ascontiguousarray(x[b].T).astype(bf)       # [C, T]

    def rows(sec, h):
        return w_attn[sec * C + h * D : sec * C + (h + 1) * D]

    q_e = np.concatenate([rows(0, h)[0::2] for h in heads])   # [192, C]
    q_o = np.concatenate([rows(0, h)[1::2] for h in heads])
    k_e = np.concatenate([rows(1, h)[0::2] for h in heads])
    k_o = np.concatenate([rows(1, h)[1::2] for h in heads])
    v_r = np.concatenate([rows(2, h) for h in heads])         # [384, C]
    wqkv = np.concatenate([q_e, q_o, k_e, k_o, v_r])          # [1152, C]
    wqkvT = np.ascontiguousarray(wqkv.T).astype(bf)           # [C, 1152]

    wp_flat = np.concatenate(
        [w_proj[:, h * D : (h + 1) * D].T for h in heads]
    )                                                         # [384, C], (h,d)-major
    wpT = np.ascontiguousarray(
        wp_flat.reshape(3, 128, C).transpose(1, 0, 2)
    ).astype(bf)                                              # [128, 3, C]
    return {"xT": xTh, "wqkvT": wqkvT, "wpT": wpT}


_NC_CACHE = {}


def _get_nc(reps=1):
    if reps not in _NC_CACHE:
        _NC_CACHE[reps] = _build_kernel(reps)
    return _NC_CACHE[reps]


def make_in_maps(x, w_attn, w_proj):
    x = np.asarray(x, np.float32)
    w_attn = np.asarray(w_attn, np.float32)
    w_proj = np.asarray(w_proj, np.float32)
    tab3 = _rope_tables()
    tm = _tri_mask()
    in_maps = []
    for core in range(NCORES):
        m = _core_inputs(x, w_attn, w_proj, core)
        m["tab3"] = tab3
        m["tm"] = tm
        in_maps.append(m)
    return in_maps


def combine_outputs(results):
    y = np.empty((B, T, C), np.float32)
    for b in range(B):
        parts = [
            results[b * HPC + g]["yp"].astype(np.float32) for g in range(HPC)
        ]
        y[b] = parts[0] + parts[1] + parts[2] + parts[3]
    return y


def kernel(x, w_attn, w_proj, _trace=False, _trace_kwargs=None):
    nc = _get_nc()
    in_maps = make_in_maps(x, w_attn, w_proj)
    res = bass_utils.run_bass_kernel_spmd(
        nc,
        in_maps,
        core_ids=list(range(NCORES)),
        trace=_trace,
        **(_trace_kwargs or {}),
    )
    out = combine_outputs(res.results)
    if _trace:
        kernel._last_results = res
    return out
